# revision 13
# baseline (speedup 1.0000x reference)
"""Trainium2 Bass kernel for nn_PhaserModel: time-varying 4-stage all-pass
phaser driven by an MLP-shaped LFO.

Strategy (8 NeuronCores = 2 stereo channels x 4 time-quarters):
  - The per-sample all-pass coefficient p[t] is a smooth function of the LFO
    phase (rate < 1 Hz).  Each core evaluates cos + the 1x32x32x1 tanh MLP on
    device at a coarse grid (one point per S=256 samples), maps the MLP
    output m through a = -tanh(tan(pi/4 - d(m))) using a degree-4 polynomial
    (host-fitted over a certified padded m-range; fp32 error ~3e-7), and
    linearly interpolates a to per-sample resolution on device.
  - The 4 cascaded all-pass stages y[n] = p[n]*s[n] + s[n-1] - p[n]*y[n-1]
    are first-order linear recurrences y[n] = a[n]*y[n-1] + b[n] with
    b[n] = s[n-1] - a[n]*s[n]; they run on the DVE's native
    tensor_tensor_scan.  Each of the 128 partitions scans its own contiguous
    512-sample chunk plus a 256-sample warmup overlap with zero initial
    state: |p| stays well below 1 (~0.27 here), so the warmup error decays
    to nothing and no cross-partition/cross-core carries are needed.
  - The scan rows are processed in two column chunks (512+256) chained via
    the scan's initial-state operand; the b-prep multiplies run on GpSimd so
    they pipeline against the DVE's scans across chunks/stages.
"""

import numpy as np

import concourse.bass as bass
import concourse.bacc as bacc
import concourse.mybir as mybir
import concourse.tile as tile
from concourse import bass_utils

SR = 44100.0
T = 262144
NCORES = 8
QT = T // 4          # output samples per core (time quarter)
P = 128              # SBUF partitions
L = QT // P          # own samples per partition = 512
W = 256              # warmup samples per partition
ROW = W + L          # scanned row length = 768
S = 256              # coarse grid spacing (samples)
NI = ROW // S        # coarse intervals per row = 3
NCC = QT // S + 2    # coarse points per core = 258
DEG = 4              # composite polynomial degree
HB = 2 * S           # column-chunk boundary (512); chunks [0,HB) [HB,ROW)
F32 = mybir.dt.float32
MMN = 512            # max matmul free dim

# spack layout: [33, 36+NCC]
SP_SC = 36           # row 32, cols 36..: rate, phi, amp
SP_G = 40            # row 0, cols 40..40+NCC: coarse grid
SP_COLS = SP_G + NCC
# frac tensor: [P, ROW + DEG + 1] — frac then broadcast poly coeffs
FR_CF = ROW
FR_COLS = ROW + DEG + 1


def _ap(t_ap, pattern, extra_offset=0):
    """Custom [step,count] access pattern on an existing AP's tensor."""
    return bass.AP(t_ap.tensor, t_ap.offset + extra_offset, pattern)


def build_program():
    Alu = mybir.AluOpType
    AF = mybir.ActivationFunctionType

    nc = bacc.Bacc(
        "TRN2", target_bir_lowering=False, debug=False, num_devices=NCORES
    )
    x_d = nc.dram_tensor("x_ext", [1, W + QT], F32, kind="ExternalInput")
    s_d = nc.dram_tensor("spack", [33, SP_COLS], F32, kind="ExternalInput")
    f_d = nc.dram_tensor("fpack", [P, FR_COLS], F32, kind="ExternalInput")
    o_d = nc.dram_tensor("out", [1, QT], F32, kind="ExternalOutput")

    with tile.TileContext(nc) as tc:
        with (
            tc.tile_pool(name="sb", bufs=1) as sb,
            tc.tile_pool(name="ps", bufs=1, space=bass.MemorySpace.PSUM) as ps,
        ):
            # ---- warm the Sin activation table before data arrives ---------
            dum = sb.tile([1, 1], F32, tag="dum")
            nc.vector.memset(dum[:], 0.0)
            dumo = sb.tile([1, 1], F32, tag="dumo")
            nc.scalar.activation(dumo[:], dum[:], AF.Sin)

            # ---- input DMAs (small consts first; x on gpsimd queue) --------
            sp = sb.tile([33, SP_COLS], F32, tag="sp")
            nc.sync.dma_start(sp[:], s_d.ap())
            fp = sb.tile([P, FR_COLS], F32, tag="fp")
            nc.sync.dma_start(fp[:], f_d.ap())
            x_ov = sb.tile([P, ROW], F32, tag="x_ov")
            nc.gpsimd.dma_start(x_ov[:], _ap(x_d.ap(), [[L, P], [1, ROW]]))

            W2ap = sp[0:32, 0:32]
            b2ap = sp[0:32, 32:33]
            W3ap = sp[0:32, 33:34]
            b1ap = sp[0:32, 34:35]
            W1ap = sp[32:33, 0:32]
            rate = sp[32:33, SP_SC : SP_SC + 1]
            phi = sp[32:33, SP_SC + 1 : SP_SC + 2]
            amp = sp[32:33, SP_SC + 2 : SP_SC + 3]
            g_ap = sp[0:1, SP_G : SP_G + NCC]
            frac = fp[:, 0:ROW]

            def cf_ap(k):  # poly coeff cf[k] (highest-first), [128,1]
                return fp[:, FR_CF + k : FR_CF + k + 1]

            # ---- tiny scalar prep ------------------------------------------
            step = sb.tile([1, 1], F32, tag="step")
            nc.vector.tensor_scalar_mul(step[:], rate, 2.0 * np.pi / SR)
            sinb = sb.tile([1, 1], F32, tag="sinb")
            nc.vector.tensor_scalar(
                sinb[:], step[:], phi, np.pi / 2, Alu.add, Alu.add
            )
            w1s = sb.tile([1, 32], F32, tag="w1s")
            nc.vector.tensor_scalar_mul(w1s[:], W1ap, amp)

            # x_half precompute (off critical path, frees the tail)
            xh = sb.tile([P, L], F32, tag="xh")
            nc.vector.tensor_scalar_mul(xh[:], x_ov[:, W:ROW], 0.5)

            # ---- coarse pipeline: cos -> MLP -> m ---------------------------
            cosr = sb.tile([1, NCC], F32, tag="cosr")
            nc.scalar.activation(
                cosr[:], g_ap, AF.Sin, bias=sinb[:], scale=step[:]
            )
            ps1 = ps.tile([32, NCC], F32, tag="ps1")
            nc.tensor.matmul(ps1[:], w1s[:], cosr[:], start=True, stop=True)
            h1 = sb.tile([32, NCC], F32, tag="h1")
            nc.scalar.activation(h1[:], ps1[:], AF.Tanh, bias=b1ap)
            ps2 = ps.tile([32, NCC], F32, tag="ps2")
            nc.tensor.matmul(ps2[:], W2ap, h1[:], start=True, stop=True)
            h2 = sb.tile([32, NCC], F32, tag="h2")
            nc.scalar.activation(h2[:], ps2[:], AF.Tanh, bias=b2ap)
            ps3 = ps.tile([1, NCC], F32, tag="ps3")
            nc.tensor.matmul(ps3[:], W3ap, h2[:], start=True, stop=True)

            # ---- redistribute m into per-partition windows ------------------
            # m_t[p, i] = m[2p + i], i in [0, NI+1)
            m_row = sb.tile([1, NCC], F32, tag="m_row")
            nc.scalar.activation(m_row[:], ps3[:], AF.Copy)
            m_t = sb.tile([P, NI + 1], F32, tag="m_t")
            nc.gpsimd.dma_start(
                m_t[:], _ap(m_row[:], [[1, 1], [L // S, P], [1, NI + 1]])
            )

            # ---- composite poly a(m), Horner sans constant term ------------
            acc = sb.tile([P, NI + 1], F32, tag="acc")
            nc.vector.tensor_scalar_mul(acc[:], m_t[:], cf_ap(0))
            for k in range(1, DEG):
                nc.vector.scalar_tensor_tensor(
                    acc[:], acc[:], cf_ap(k), m_t[:], Alu.add, Alu.mult
                )
            dlt = sb.tile([P, NI], F32, tag="dlt")
            nc.vector.tensor_sub(dlt[:], acc[:, 1 : NI + 1], acc[:, 0:NI])

            # ---- upsample per column chunk: a = (acc + cf[DEG]) + dlt*frac -
            a_ov = sb.tile([P, ROW], F32, tag="a_ov")
            a3 = a_ov[:].rearrange("p (c s) -> p c s", s=S)
            f3 = frac.rearrange("p (c s) -> p c s", s=S)
            chunks = [(0, HB), (HB, ROW)]
            for c0, c1 in chunks:
                i0, i1 = c0 // S, c1 // S
                dlt_b = dlt[:, i0:i1].unsqueeze(2).broadcast_to(
                    (P, i1 - i0, S)
                )
                acc_b = acc[:, i0:i1].unsqueeze(2).broadcast_to(
                    (P, i1 - i0, S)
                )
                nc.vector.tensor_tensor(
                    a3[:, i0:i1], f3[:, i0:i1], dlt_b, Alu.mult
                )
                nc.vector.scalar_tensor_tensor(
                    a3[:, i0:i1], acc_b, cf_ap(DEG), a3[:, i0:i1],
                    Alu.add, Alu.add,
                )

            # ---- 4 cascaded all-pass stages, chunk-pipelined ----------------
            # b-prep multiply on GpSimd; shifted subtract + scan on DVE.
            s_cur = x_ov
            for k in range(4):
                tmp = sb.tile([P, ROW], F32, tag=f"tmp{k}")
                y = sb.tile([P, ROW], F32, tag=f"y{k}")
                for c0, c1 in chunks:
                    nc.gpsimd.tensor_tensor(
                        tmp[:, c0:c1], a_ov[:, c0:c1], s_cur[:, c0:c1],
                        Alu.mult,
                    )
                    # b[:, j] = s[:, j-1] - (a*s)[:, j]   (col 0 of chunk 0
                    # left as a*s garbage; decays within the warmup)
                    lo = max(c0, 1)
                    nc.vector.tensor_tensor(
                        tmp[:, lo:c1], s_cur[:, lo - 1 : c1 - 1],
                        tmp[:, lo:c1], Alu.subtract,
                    )
                    nc.vector.tensor_tensor_scan(
                        y[:, c0:c1], a_ov[:, c0:c1], tmp[:, c0:c1],
                        0.0 if c0 == 0 else y[:, c0 - 1 : c0],
                        Alu.mult, Alu.add,
                    )
                s_cur = y

            # ---- dry/wet mix + store, chunked: out = 0.5*y4 + xh -----------
            osb = sb.tile([P, L], F32, tag="osb")
            for c0, c1 in chunks:
                o0, o1 = max(c0, W) - W, c1 - W
                nc.vector.scalar_tensor_tensor(
                    osb[:, o0:o1], s_cur[:, o0 + W : c1], 0.5, xh[:, o0:o1],
                    Alu.mult, Alu.add,
                )
                nc.sync.dma_start(
                    _ap(o_d.ap(), [[L, P], [1, o1 - o0]], extra_offset=o0),
                    osb[:, o0:o1],
                )

    nc.compile()
    return nc


def _fit_composite(lfo_rate, off, amp, bias, depth, W1, b1, W2, b2, W3, b3):
    """Host-side: certify the m-range via a coarse probe and fit the
    degree-DEG polynomial for a(m) = -tanh(tan(pi/4 - d(m))) in raw m.
    Only O(1k) scalar work independent of T."""
    W1, b1, W2, b2, W3, b3 = [
        np.asarray(v, np.float64) for v in (W1, b1, W2, b2, W3, b3)
    ]
    rate = float(np.asarray(lfo_rate).reshape(-1)[0])
    amp, bias, depth = (float(np.asarray(v)) for v in (amp, bias, depth))
    b3v = float(b3.reshape(-1)[0])
    c1 = -depth / 2.0
    zb = np.pi / 4 - bias - depth / 2.0 + c1 * b3v
    step = 2.0 * np.pi * rate / SR
    n = np.linspace(0.0, T, 1025)
    ms = []
    for phi in (0.0, float(np.asarray(off).reshape(-1)[0])):
        lfo = amp * np.cos((n + 1.0) * step + phi)
        h = np.tanh(lfo[:, None] @ W1.reshape(1, 32) + b1.reshape(32))
        h = np.tanh(h @ W2 + b2.reshape(32))
        ms.append((h @ W3.reshape(32, 1))[:, 0])
    ms = np.concatenate(ms)
    pad = 0.3 + 0.1 * (ms.max() - ms.min())
    mlo, mhi = ms.min() - pad, ms.max() + pad
    wlo, whi = sorted((c1 * mlo + zb, c1 * mhi + zb))
    assert -1.55 < wlo and whi < 1.55, f"tan arg out of range: {wlo},{whi}"
    m = np.linspace(mlo, mhi, 4001)
    a_true = -np.tanh(np.tan(c1 * m + zb))
    cf = np.polyfit(m, a_true, DEG)
    fit_err = np.abs(np.polyval(cf, m) - a_true).max()
    assert fit_err < 5e-6, f"poly fit error too large: {fit_err}"
    return cf


def make_in_maps(x, lfo_rate, lfo_stereo_phase_offset, amp, bias, depth,
                 W1, b1, W2, b2, W3, b3):
    x = np.asarray(x, np.float32).reshape(-1)
    off = np.asarray(lfo_stereo_phase_offset, np.float32).reshape(-1)[0]
    cf = _fit_composite(
        lfo_rate, off, amp, bias, depth, W1, b1, W2, b2, W3, b3
    )
    fpack = np.zeros((P, FR_COLS), np.float32)
    fpack[:, 0:ROW] = ((np.arange(ROW) % S) / S).astype(np.float32)
    for k in range(DEG + 1):
        fpack[:, FR_CF + k] = np.float32(cf[k])
    in_maps = []
    for core in range(NCORES):
        ch, q = divmod(core, 4)
        T0 = QT * q
        if T0 - W >= 0:
            x_ext = x[T0 - W : T0 + QT]
        else:
            x_ext = np.concatenate([np.zeros(W, np.float32), x[0 : T0 + QT]])
        spack = np.zeros((33, SP_COLS), np.float32)
        spack[0:32, 0:32] = np.asarray(W2, np.float32)
        spack[0:32, 32] = np.asarray(b2, np.float32).reshape(32)
        spack[0:32, 33] = np.asarray(W3, np.float32).reshape(32)
        spack[0:32, 34] = np.asarray(b1, np.float32).reshape(32)
        spack[32, 0:32] = np.asarray(W1, np.float32).reshape(32)
        spack[32, SP_SC + 0] = np.float32(np.asarray(lfo_rate).reshape(-1)[0])
        spack[32, SP_SC + 1] = np.float32(0.0 if ch == 0 else off)
        spack[32, SP_SC + 2] = np.float32(np.asarray(amp))
        spack[0, SP_G : SP_G + NCC] = (
            (np.arange(NCC, dtype=np.float64) + (T0 // S) - 1) * S
        ).astype(np.float32)
        in_maps.append(
            {"x_ext": x_ext.reshape(1, W + QT).copy(), "spack": spack,
             "fpack": fpack}
        )
    return in_maps


_prog_cache = {}


def kernel(**inputs) -> np.ndarray:
    if "nc" not in _prog_cache:
        _prog_cache["nc"] = build_program()
    nc = _prog_cache["nc"]
    in_maps = make_in_maps(**inputs)
    res = bass_utils.run_bass_kernel_spmd(
        nc, in_maps, core_ids=list(range(NCORES))
    )
    out = np.empty((2, T), np.float32)
    for core in range(NCORES):
        ch, q = divmod(core, 4)
        out[ch, QT * q : QT * (q + 1)] = res.results[core]["out"][0]
    return out


# revision 16
# speedup vs baseline: 1.1890x; 1.1890x over previous
"""Trainium2 Bass kernel for nn_PhaserModel: time-varying 4-stage all-pass
phaser driven by an MLP-shaped LFO.

Strategy (8 NeuronCores = 2 stereo channels x 4 time-quarters):
  - The per-sample all-pass coefficient p[t] is a smooth function of the LFO
    phase (rate < 1 Hz).  Each core evaluates cos + the 1x32x32x1 tanh MLP on
    device at a coarse grid (one point per S=256 samples), maps the MLP
    output m through a = -tanh(tan(pi/4 - d(m))) using a degree-4 polynomial
    (host-fitted over a certified padded m-range; fp32 error ~3e-7), and
    linearly interpolates a to per-sample resolution on device.
  - The 4 cascaded all-pass stages y[n] = p[n]*s[n] + s[n-1] - p[n]*y[n-1]
    are first-order linear recurrences y[n] = a[n]*y[n-1] + b[n] with
    b[n] = s[n-1] - a[n]*s[n]; they run on the DVE's native
    tensor_tensor_scan.  Each of the 128 partitions scans its own contiguous
    512-sample chunk plus a 256-sample warmup overlap with zero initial
    state: |p| stays well below 1 (~0.27 here), so the warmup error decays
    to nothing and no cross-partition/cross-core carries are needed.
  - The scan rows are processed in two column chunks (512+256) chained via
    the scan's initial-state operand; the b-prep multiplies run on GpSimd so
    they pipeline against the DVE's scans across chunks/stages.
"""

import numpy as np

import concourse.bass as bass
import concourse.bacc as bacc
import concourse.mybir as mybir
import concourse.tile as tile
from concourse import bass_utils

SR = 44100.0
T = 262144
NCORES = 8
QT = T // 4          # output samples per core (time quarter)
P = 128              # SBUF partitions
L = QT // P          # own samples per partition = 512
W = 64               # warmup samples per partition (|p|~0.27 -> 1e-36 decay)
ROW = W + L          # scanned row length = 576
S = 256              # coarse grid spacing (samples); must divide L
NW = 4               # coarse window length per partition (3 intervals + end)
NCC = QT // S + 2    # coarse points per core = 258
DEG = 4              # composite polynomial degree
F32 = mybir.dt.float32
MMN = 512            # max matmul free dim

# spack layout: [33, 36+NCC]
SP_SC = 36           # row 32, cols 36..: rate, phi, amp
SP_G = 40            # row 0, cols 40..40+NCC: coarse grid
SP_COLS = SP_G + NCC
# frac tensor: [P, ROW + DEG + 1] — frac then broadcast poly coeffs
FR_CF = ROW
FR_COLS = ROW + DEG + 1


def _ap(t_ap, pattern, extra_offset=0):
    """Custom [step,count] access pattern on an existing AP's tensor."""
    return bass.AP(t_ap.tensor, t_ap.offset + extra_offset, pattern)


def build_program():
    Alu = mybir.AluOpType
    AF = mybir.ActivationFunctionType

    nc = bacc.Bacc(
        "TRN2", target_bir_lowering=False, debug=False, num_devices=NCORES
    )
    x_d = nc.dram_tensor("x_ext", [1, W + QT], F32, kind="ExternalInput")
    s_d = nc.dram_tensor("spack", [33, SP_COLS], F32, kind="ExternalInput")
    f_d = nc.dram_tensor("fpack", [P, FR_COLS], F32, kind="ExternalInput")
    o_d = nc.dram_tensor("out", [1, QT], F32, kind="ExternalOutput")

    with tile.TileContext(nc) as tc:
        with (
            tc.tile_pool(name="sb", bufs=1) as sb,
            tc.tile_pool(name="ps", bufs=1, space=bass.MemorySpace.PSUM) as ps,
        ):
            # ---- warm the Sin activation table before data arrives ---------
            dum = sb.tile([1, 1], F32, tag="dum")
            nc.vector.memset(dum[:], 0.0)
            dumo = sb.tile([1, 1], F32, tag="dumo")
            nc.scalar.activation(dumo[:], dum[:], AF.Sin)

            # ---- input DMAs (small consts first; x on gpsimd queue) --------
            sp = sb.tile([33, SP_COLS], F32, tag="sp")
            nc.sync.dma_start(sp[:], s_d.ap())
            fp = sb.tile([P, FR_COLS], F32, tag="fp")
            nc.sync.dma_start(fp[:], f_d.ap())
            x_ov = sb.tile([P, ROW], F32, tag="x_ov")
            nc.gpsimd.dma_start(x_ov[:], _ap(x_d.ap(), [[L, P], [1, ROW]]))

            W2ap = sp[0:32, 0:32]
            b2ap = sp[0:32, 32:33]
            W3ap = sp[0:32, 33:34]
            b1ap = sp[0:32, 34:35]
            W1ap = sp[32:33, 0:32]
            rate = sp[32:33, SP_SC : SP_SC + 1]
            phi = sp[32:33, SP_SC + 1 : SP_SC + 2]
            amp = sp[32:33, SP_SC + 2 : SP_SC + 3]
            g_ap = sp[0:1, SP_G : SP_G + NCC]
            frac = fp[:, 0:ROW]

            def cf_ap(k):  # poly coeff cf[k] (highest-first), [128,1]
                return fp[:, FR_CF + k : FR_CF + k + 1]

            # ---- tiny scalar prep ------------------------------------------
            step = sb.tile([1, 1], F32, tag="step")
            nc.vector.tensor_scalar_mul(step[:], rate, 2.0 * np.pi / SR)
            sinb = sb.tile([1, 1], F32, tag="sinb")
            nc.vector.tensor_scalar(
                sinb[:], step[:], phi, np.pi / 2, Alu.add, Alu.add
            )
            w1s = sb.tile([1, 32], F32, tag="w1s")
            nc.vector.tensor_scalar_mul(w1s[:], W1ap, amp)

            # x_half precompute (off critical path, frees the tail)
            xh = sb.tile([P, L], F32, tag="xh")
            nc.vector.tensor_scalar_mul(xh[:], x_ov[:, W:ROW], 0.5)

            # ---- coarse pipeline: cos -> MLP -> m ---------------------------
            cosr = sb.tile([1, NCC], F32, tag="cosr")
            nc.scalar.activation(
                cosr[:], g_ap, AF.Sin, bias=sinb[:], scale=step[:]
            )
            ps1 = ps.tile([32, NCC], F32, tag="ps1")
            nc.tensor.matmul(ps1[:], w1s[:], cosr[:], start=True, stop=True)
            h1 = sb.tile([32, NCC], F32, tag="h1")
            nc.scalar.activation(h1[:], ps1[:], AF.Tanh, bias=b1ap)
            ps2 = ps.tile([32, NCC], F32, tag="ps2")
            nc.tensor.matmul(ps2[:], W2ap, h1[:], start=True, stop=True)
            h2 = sb.tile([32, NCC], F32, tag="h2")
            nc.scalar.activation(h2[:], ps2[:], AF.Tanh, bias=b2ap)
            ps3 = ps.tile([1, NCC], F32, tag="ps3")
            nc.tensor.matmul(ps3[:], W3ap, h2[:], start=True, stop=True)

            # ---- redistribute m into per-partition windows ------------------
            # m_t[p, i] = m[2p + i], i in [0, NW)
            m_row = sb.tile([1, NCC], F32, tag="m_row")
            nc.scalar.activation(m_row[:], ps3[:], AF.Copy)
            m_t = sb.tile([P, NW], F32, tag="m_t")
            nc.gpsimd.dma_start(
                m_t[:], _ap(m_row[:], [[1, 1], [L // S, P], [1, NW]])
            )

            # ---- composite poly a(m), Horner sans constant term ------------
            acc = sb.tile([P, NW], F32, tag="acc")
            nc.vector.tensor_scalar_mul(acc[:], m_t[:], cf_ap(0))
            for k in range(1, DEG):
                nc.vector.scalar_tensor_tensor(
                    acc[:], acc[:], cf_ap(k), m_t[:], Alu.add, Alu.mult
                )
            dlt = sb.tile([P, NW - 1], F32, tag="dlt")
            nc.vector.tensor_sub(dlt[:], acc[:, 1:NW], acc[:, 0 : NW - 1])

            # ---- upsample: a = (acc + cf[DEG]) + dlt*frac ------------------
            # view 1: partial first interval, cols [0, W) (window 0)
            # view 2: full intervals, cols [W, ROW) as [P, 2, S] (windows 1,2)
            a_ov = sb.tile([P, ROW], F32, tag="a_ov")
            d0b = dlt[:, 0:1].broadcast_to((P, W))
            a0b = acc[:, 0:1].broadcast_to((P, W))
            nc.vector.tensor_tensor(
                a_ov[:, 0:W], frac[:, 0:W], d0b, Alu.mult
            )
            nc.vector.scalar_tensor_tensor(
                a_ov[:, 0:W], a0b, cf_ap(DEG), a_ov[:, 0:W], Alu.add, Alu.add
            )
            a3 = a_ov[:, W:ROW].rearrange("p (c s) -> p c s", s=S)
            f3 = frac[:, W:ROW].rearrange("p (c s) -> p c s", s=S)
            dlt_b = dlt[:, 1:3].unsqueeze(2).broadcast_to((P, 2, S))
            acc_b = acc[:, 1:3].unsqueeze(2).broadcast_to((P, 2, S))
            nc.vector.tensor_tensor(a3, f3, dlt_b, Alu.mult)
            nc.vector.scalar_tensor_tensor(
                a3, acc_b, cf_ap(DEG), a3, Alu.add, Alu.add
            )

            # ---- 4 cascaded all-pass stages via native scan ----------------
            s_cur = x_ov
            for k in range(4):
                tmp = sb.tile([P, ROW], F32, tag=f"tmp{k}")
                nc.vector.tensor_mul(tmp[:], a_ov[:], s_cur[:])
                # b[:,1:] = s[:, :-1] - (a*s)[:, 1:]  (col 0 garbage; decays)
                nc.vector.tensor_tensor(
                    tmp[:, 1:ROW], s_cur[:, 0 : ROW - 1], tmp[:, 1:ROW],
                    Alu.subtract,
                )
                y = sb.tile([P, ROW], F32, tag=f"y{k}")
                nc.vector.tensor_tensor_scan(
                    y[:], a_ov[:], tmp[:], 0.0, Alu.mult, Alu.add
                )
                s_cur = y

            # ---- dry/wet mix + store: out = 0.5*y4 + xh --------------------
            osb = sb.tile([P, L], F32, tag="osb")
            nc.vector.scalar_tensor_tensor(
                osb[:], s_cur[:, W:ROW], 0.5, xh[:], Alu.mult, Alu.add
            )
            nc.sync.dma_start(_ap(o_d.ap(), [[L, P], [1, L]]), osb[:])

    nc.compile()
    return nc


def _fit_composite(lfo_rate, off, amp, bias, depth, W1, b1, W2, b2, W3, b3):
    """Host-side: certify the m-range via a coarse probe and fit the
    degree-DEG polynomial for a(m) = -tanh(tan(pi/4 - d(m))) in raw m.
    Only O(1k) scalar work independent of T."""
    W1, b1, W2, b2, W3, b3 = [
        np.asarray(v, np.float64) for v in (W1, b1, W2, b2, W3, b3)
    ]
    rate = float(np.asarray(lfo_rate).reshape(-1)[0])
    amp, bias, depth = (float(np.asarray(v)) for v in (amp, bias, depth))
    b3v = float(b3.reshape(-1)[0])
    c1 = -depth / 2.0
    zb = np.pi / 4 - bias - depth / 2.0 + c1 * b3v
    step = 2.0 * np.pi * rate / SR
    n = np.linspace(0.0, T, 1025)
    ms = []
    for phi in (0.0, float(np.asarray(off).reshape(-1)[0])):
        lfo = amp * np.cos((n + 1.0) * step + phi)
        h = np.tanh(lfo[:, None] @ W1.reshape(1, 32) + b1.reshape(32))
        h = np.tanh(h @ W2 + b2.reshape(32))
        ms.append((h @ W3.reshape(32, 1))[:, 0])
    ms = np.concatenate(ms)
    pad = 0.3 + 0.1 * (ms.max() - ms.min())
    mlo, mhi = ms.min() - pad, ms.max() + pad
    wlo, whi = sorted((c1 * mlo + zb, c1 * mhi + zb))
    assert -1.55 < wlo and whi < 1.55, f"tan arg out of range: {wlo},{whi}"
    m = np.linspace(mlo, mhi, 4001)
    a_true = -np.tanh(np.tan(c1 * m + zb))
    cf = np.polyfit(m, a_true, DEG)
    fit_err = np.abs(np.polyval(cf, m) - a_true).max()
    assert fit_err < 5e-6, f"poly fit error too large: {fit_err}"
    return cf


def make_in_maps(x, lfo_rate, lfo_stereo_phase_offset, amp, bias, depth,
                 W1, b1, W2, b2, W3, b3):
    x = np.asarray(x, np.float32).reshape(-1)
    off = np.asarray(lfo_stereo_phase_offset, np.float32).reshape(-1)[0]
    cf = _fit_composite(
        lfo_rate, off, amp, bias, depth, W1, b1, W2, b2, W3, b3
    )
    fpack = np.zeros((P, FR_COLS), np.float32)
    fpack[:, 0:ROW] = (((np.arange(ROW) - W) % S) / S).astype(np.float32)
    for k in range(DEG + 1):
        fpack[:, FR_CF + k] = np.float32(cf[k])
    in_maps = []
    for core in range(NCORES):
        ch, q = divmod(core, 4)
        T0 = QT * q
        if T0 - W >= 0:
            x_ext = x[T0 - W : T0 + QT]
        else:
            x_ext = np.concatenate([np.zeros(W, np.float32), x[0 : T0 + QT]])
        spack = np.zeros((33, SP_COLS), np.float32)
        spack[0:32, 0:32] = np.asarray(W2, np.float32)
        spack[0:32, 32] = np.asarray(b2, np.float32).reshape(32)
        spack[0:32, 33] = np.asarray(W3, np.float32).reshape(32)
        spack[0:32, 34] = np.asarray(b1, np.float32).reshape(32)
        spack[32, 0:32] = np.asarray(W1, np.float32).reshape(32)
        spack[32, SP_SC + 0] = np.float32(np.asarray(lfo_rate).reshape(-1)[0])
        spack[32, SP_SC + 1] = np.float32(0.0 if ch == 0 else off)
        spack[32, SP_SC + 2] = np.float32(np.asarray(amp))
        spack[0, SP_G : SP_G + NCC] = (
            (np.arange(NCC, dtype=np.float64) + (T0 // S) - 1) * S
        ).astype(np.float32)
        in_maps.append(
            {"x_ext": x_ext.reshape(1, W + QT).copy(), "spack": spack,
             "fpack": fpack}
        )
    return in_maps


_prog_cache = {}


def kernel(**inputs) -> np.ndarray:
    if "nc" not in _prog_cache:
        _prog_cache["nc"] = build_program()
    nc = _prog_cache["nc"]
    in_maps = make_in_maps(**inputs)
    res = bass_utils.run_bass_kernel_spmd(
        nc, in_maps, core_ids=list(range(NCORES))
    )
    out = np.empty((2, T), np.float32)
    for core in range(NCORES):
        ch, q = divmod(core, 4)
        out[ch, QT * q : QT * (q + 1)] = res.results[core]["out"][0]
    return out
